# revision 35
# baseline (speedup 1.0000x reference)
"""GCN layer kernel for Trainium2 (8 NeuronCores).

Strategy:
  - Nodes assigned to 8 cores x BPC blocks of 128 via balanced packing so
    each (block, src-quarter) edge segment fits 4*128 edges (minimal pad).
  - Edge features: per (block, quarter) segment of <=512 edges, the first
    256 edge slots are served from a host-built per-block halo table
    (dense, pre-transposed, one HWDGE dma_start per group-quarter); the
    remaining <=256 slots are fetched with gpsimd.dma_gather (SWDGE).
    This halves the SWDGE call count (calls are latency-bound: only 4 fly
    concurrently at ~9.5us each), moving half the traffic to the fast
    dense path per the sharding hint's halo-copy scheme.
  - Segment-sum: per 128-edge chunk, one-hot S[e, slot] built on DVE
    (is_equal vs iota, batched per block), PE matmul accumulates
    psum[feat, slot] += E_bf16^T S over the block's chunks. Pad edges get
    slot 128 (PSUM cols 128..131 ignored).
  - mean+linear+relu+residual: psum1 -> SBUF (ACT copy), f32 matmul with
    W^T, ACT relu with per-partition scale 1/deg (folds the mean), DVE
    residual add, DMA out. deg==0 nodes get a self-edge and scale 1 so
    they keep x (DGL semantics).
"""
import sys
sys.path.insert(0, "/opt/trn_rl_repo")

import numpy as np
import ml_dtypes

import concourse.bass as bass
import concourse.mybir as mybir
import concourse.tile as tile
from concourse import bacc, bass_utils

F32 = mybir.dt.float32
BF16 = mybir.dt.bfloat16
I16 = mybir.dt.int16

N_NODES = 100000
D = 128
NCORES = 8
SW = 132  # slot one-hot width (128 real slots + pad cols)
PAD_SLOT = 128
GATHER_CAP = 1024  # dma_gather crashes above 1024 indices per call
DN = 256  # dense (halo) slots per (block, quarter); rest gathered

_BUILD_CACHE = {}
LAST_RESULTS = None  # for test harness introspection


def _plan_groups(bpc, gmax):
    groups = []
    left = bpc
    while left > 0:
        g = min(gmax, left)
        groups.append(g)
        left -= g
    return groups


def _build(c_list, groups, bpc, npad, qrows):
    """Build + compile the SPMD Bass program. Same program for all 8 cores."""
    key = (tuple(c_list), tuple(groups), bpc, npad, qrows)
    if key in _BUILD_CACHE:
        return _BUILD_CACHE[key]

    assert all(c >= 3 for c in c_list)
    csum = int(sum(c_list))
    dchunks = [c - 2 for c in c_list]  # dense chunks per (block, quarter)
    totdense = bpc * sum(dchunks) * 128
    totgather = bpc * 4 * DN
    npc = bpc * 128  # nodes per core

    nc = bacc.Bacc("TRN2", target_bir_lowering=False, debug=False,
                   num_devices=NCORES, num_swdge_queues=4)
    xq = nc.dram_tensor("xq", [npad, D], BF16, kind="ExternalInput")
    idxd = nc.dram_tensor("idxd", [128, totgather // 16], I16,
                          kind="ExternalInput")
    densed = nc.dram_tensor("densed", [128, totdense], BF16,
                            kind="ExternalInput")
    slotd = nc.dram_tensor("slotd", [128, bpc * csum], BF16,
                           kind="ExternalInput")
    invd = nc.dram_tensor("invd", [128, bpc], F32, kind="ExternalInput")
    xod = nc.dram_tensor("xod", [npc, D], F32, kind="ExternalInput")
    wtd = nc.dram_tensor("wtd", [D, D], BF16, kind="ExternalInput")
    iotad = nc.dram_tensor("iotad", [128, SW], BF16, kind="ExternalInput")
    outd = nc.dram_tensor("out", [npc, D], F32, kind="ExternalOutput")

    qcall = 0  # rotates gather calls over the 4 SWDGE queues

    with tile.TileContext(nc) as tc:
        with tc.tile_pool(name="const", bufs=1) as const, \
             tc.tile_pool(name="ework", bufs=3) as ework, \
             tc.tile_pool(name="sbwork", bufs=3) as sbwork, \
             tc.tile_pool(name="psum1", bufs=2, space="PSUM") as psum1p, \
             tc.tile_pool(name="psum2", bufs=2, space="PSUM") as psum2p:

            wt_t = const.tile([128, D], BF16)
            nc.sync.dma_start(out=wt_t[:], in_=wtd[:, :])
            iota_t = const.tile([128, SW], BF16)
            nc.sync.dma_start(out=iota_t[:], in_=iotad[:, :])
            inv_t = const.tile([128, bpc], F32)
            nc.sync.dma_start(out=inv_t[:], in_=invd[:, :])

            d_base = 0  # (g, q)-major base into densed cols
            g_base = 0  # (g, q)-major base into gather slots
            b0 = 0
            for g_i, G in enumerate(groups):
                slot_t = ework.tile([128, G * csum], BF16, tag="slot")
                nc.sync.dma_start(
                    out=slot_t[:],
                    in_=slotd[:, b0 * csum:(b0 + G) * csum])
                ed_list = []
                eg_list = []
                for q in range(4):
                    nd = G * dchunks[q] * 128
                    ng = G * DN
                    # dense halo part: one contiguous HWDGE load, issued
                    # from the (mostly idle) ACT engine so the big slabs
                    # don't clog the Sync queue ahead of the idx loads
                    ef_d = ework.tile([128, G * dchunks[q], 128], BF16,
                                      tag=f"efd{q}")
                    nc.scalar.dma_start(
                        out=ef_d[:, :, :],
                        in_=densed[:, d_base:d_base + nd])
                    # gather part: one SWDGE call of G*DN (=1024) idxs
                    ncols = ng // 16
                    idx_t = ework.tile([128, ncols], I16, tag=f"idx{q}")
                    nc.sync.dma_start(
                        out=idx_t[:],
                        in_=idxd[:, g_base // 16:
                                 g_base // 16 + ncols])
                    ef_g = ework.tile([128, G * 2, 128], BF16,
                                      tag=f"efg{q}")
                    c0 = 0
                    while c0 * 128 < ng:
                        n_call = min(GATHER_CAP, ng - c0 * 128)
                        nch = n_call // 128
                        nc.gpsimd.dma_gather(
                            out_ap=ef_g[:, c0:c0 + nch, :],
                            in_ap=xq[q * qrows:, :],
                            idxs_ap=idx_t[:, c0 * 8:(c0 + nch) * 8],
                            num_idxs=n_call,
                            num_idxs_reg=n_call,
                            elem_size=D,
                            queue_num=qcall % 4,
                        )
                        qcall += 1
                        c0 += nch
                    ed_list.append(ef_d)
                    eg_list.append(ef_g)
                    d_base += nd
                    g_base += ng

                # one-hot build batched over the whole group: one DVE
                # instruction for all G blocks amortizes instr overhead
                s_t = ework.tile([128, G * csum, SW], BF16, tag="S")
                nc.vector.tensor_tensor(
                    out=s_t[:, :, :],
                    in0=slot_t[:].unsqueeze(2).to_broadcast(
                        [128, G * csum, SW]),
                    in1=iota_t[:].unsqueeze(1).to_broadcast(
                        [128, G * csum, SW]),
                    op=mybir.AluOpType.is_equal,
                )
                for bl in range(G):
                    b = b0 + bl
                    p1 = psum1p.tile([128, SW], F32, tag="p1")
                    cglob = 0
                    for q in range(4):
                        dc = dchunks[q]
                        for k in range(c_list[q]):
                            if k < dc:
                                lhs = ed_list[q][:, bl * dc + k, :]
                            else:
                                lhs = eg_list[q][:, bl * 2 + k - dc, :]
                            nc.tensor.matmul(
                                out=p1[:, :],
                                lhsT=lhs,
                                rhs=s_t[:, bl * csum + cglob, :],
                                start=(cglob == 0),
                                stop=(cglob == csum - 1),
                            )
                            cglob += 1
                    agg_t = sbwork.tile([128, 128], BF16, tag="aggT")
                    nc.scalar.copy(agg_t[:], p1[:, 0:128])
                    p2 = psum2p.tile([128, 128], F32, tag="p2")
                    nc.tensor.matmul(out=p2[:, :], lhsT=agg_t[:],
                                     rhs=wt_t[:], start=True, stop=True)
                    xo_t = sbwork.tile([128, 128], F32, tag="xo")
                    nc.sync.dma_start(out=xo_t[:],
                                      in_=xod[b * 128:(b + 1) * 128, :])
                    hb_t = sbwork.tile([128, 128], F32, tag="hb")
                    nc.scalar.activation(
                        hb_t[:], p2[:, :],
                        mybir.ActivationFunctionType.Relu,
                        scale=inv_t[:, b:b + 1])
                    ob_t = sbwork.tile([128, 128], F32, tag="ob")
                    nc.vector.tensor_add(ob_t[:], hb_t[:], xo_t[:])
                    nc.sync.dma_start(out=outd[b * 128:(b + 1) * 128, :],
                                      in_=ob_t[:])
                b0 += G
    nc.compile()
    _BUILD_CACHE[key] = nc
    return nc


def _pack_blocks(qd, nblocks, cap, node_cap=128):
    """Greedy 4-D balanced packing: assign nodes to blocks so that each
    block's per-quarter edge counts stay <= cap and node count <= node_cap.
    qd: [n, 4] per-node per-quarter in-degree. Returns block id per node,
    or None if infeasible."""
    n = qd.shape[0]
    loads = np.zeros((nblocks, 4), dtype=np.int64)
    slots = np.full(nblocks, node_cap, dtype=np.int64)
    assign = np.full(n, -1, dtype=np.int64)
    order = np.argsort(-qd.sum(1), kind="stable")
    # big nodes first with exact argmin; tail nodes in bulk round-robin
    big = order[qd[order].sum(1) > 24]
    small = order[qd[order].sum(1) <= 24]
    for i in big:
        score = (loads + qd[i]).max(1)
        score[slots <= 0] = 1 << 40
        score[(loads + qd[i] > cap).any(1)] = 1 << 40
        b = int(np.argmin(score))
        if score[b] >= 1 << 40:
            return None
        assign[i] = b
        loads[b] += qd[i]
        slots[b] -= 1
    # small nodes: repeatedly place into least-loaded blocks
    for i in small:
        score = (loads + qd[i]).max(1).astype(np.float64)
        score += (node_cap - slots) * 1e-3  # prefer emptier blocks slightly
        score[slots <= 0] = 1e18
        score[(loads + qd[i] > cap).any(1)] = 1e18
        b = int(np.argmin(score))
        if score[b] >= 1e18:
            return None
        assign[i] = b
        loads[b] += qd[i]
        slots[b] -= 1
    return assign


def _preprocess(x, src, dst, W, n_nodes, ncores, gmax=4, bpc=None):
    """Host-side graph partitioning -> per-core tensors + halo tables."""
    D_ = x.shape[1]
    if bpc is None:
        min_bpc = -(-n_nodes // (ncores * 128))
        bpc = min_bpc + 2 if min_bpc > 8 else min_bpc

    deg = np.bincount(dst, minlength=n_nodes)
    inv = 1.0 / np.maximum(deg, 1).astype(np.float32)
    zero_deg = np.where(deg == 0)[0]
    if len(zero_deg):
        src = np.concatenate([src, zero_deg])
        dst = np.concatenate([dst, zero_deg])

    # Try packed c=[4,4,4,4] at increasing bpc (cap 512 needs mean
    # quarter load comfortably below 512 -> extra block slack), else
    # fall back to natural blocks with data-derived budgets.
    assign = None
    for bpc_try in (bpc + 4, bpc + 8):
        npad_t = bpc_try * 128 * ncores
        qrows_t = npad_t // 4
        quarter_t = (src // qrows_t).astype(np.int64)
        qd = np.zeros((npad_t, 4), dtype=np.int64)
        np.add.at(qd, (dst, quarter_t), 1)
        assign = _pack_blocks(qd, ncores * bpc_try, cap=512)
        if assign is not None:
            bpc = bpc_try
            break
    npc = bpc * 128
    npad = npc * ncores
    qrows = npad // 4
    nblocks = ncores * bpc
    quarter = (src // qrows).astype(np.int64)
    if assign is not None:
        c_list = [4, 4, 4, 4]
    else:
        # fallback: natural-order blocks, budgets from data (+2 gather
        # chunks are carved from each budget, so need c >= 3)
        assign = np.arange(npad) // 128
        cmat = np.zeros((nblocks, 4), dtype=np.int64)
        np.add.at(cmat, (assign[dst], quarter), 1)
        c_list = [max(int(-(-cmat[:, q].max() // 128)), 3) for q in range(4)]
    csum = int(sum(c_list))
    dchunks = [c - 2 for c in c_list]
    qoff = np.concatenate([[0], np.cumsum(c_list)]).astype(int)

    # node -> (block, slot); slot = rank within block
    blk_of = assign[:npad].copy()
    unassigned = np.where(blk_of < 0)[0]
    if len(unassigned):
        counts = np.bincount(blk_of[blk_of >= 0], minlength=nblocks)
        free = []
        for b in range(nblocks):
            free.extend([b] * (128 - counts[b]))
        blk_of[unassigned] = np.array(free[:len(unassigned)], dtype=np.int64)
    order_nodes = np.argsort(blk_of, kind="stable")
    slot_of = np.zeros(npad, dtype=np.int64)
    counts = np.bincount(blk_of, minlength=nblocks)
    assert counts.max() <= 128, "block overflow"
    start = np.concatenate([[0], np.cumsum(counts)])
    slot_of[order_nodes] = np.arange(npad) - start[blk_of[order_nodes]]
    pos_of = blk_of * 128 + slot_of
    perm = np.zeros(npad, dtype=np.int64)
    perm[pos_of] = np.arange(npad)

    groups = _plan_groups(bpc, gmax)
    # (g, q)-major bases for dense slots and gather slots
    d_b = np.zeros((len(groups), 4), dtype=np.int64)
    g_b = np.zeros((len(groups), 4), dtype=np.int64)
    dpos_acc = gpos_acc = 0
    for gi, G in enumerate(groups):
        for q in range(4):
            d_b[gi, q] = dpos_acc
            g_b[gi, q] = gpos_acc
            dpos_acc += G * dchunks[q] * 128
            gpos_acc += G * DN
    totdense, totgather = dpos_acc, gpos_acc
    assert totgather == bpc * 4 * DN

    g_of_block = np.zeros(bpc, dtype=np.int64)
    boff_of_block = np.zeros(bpc, dtype=np.int64)
    b = 0
    for gi, G in enumerate(groups):
        for j in range(G):
            g_of_block[b] = gi
            boff_of_block[b] = j
            b += 1

    # per-edge data
    blk_e = blk_of[dst]
    slot_e = slot_of[dst]
    order = np.lexsort((src, quarter, blk_e))
    src_s = src[order]
    q_s = quarter[order]
    blk_s = blk_e[order]
    slot_s = slot_e[order]

    seg_id = blk_s * 4 + q_s
    seg_counts = np.bincount(seg_id, minlength=nblocks * 4)
    cmat2 = seg_counts.reshape(nblocks, 4)
    for q in range(4):
        assert cmat2[:, q].max() <= c_list[q] * 128, \
            f"quarter {q} overflow: {cmat2[:, q].max()}"
    seg_start = np.concatenate([[0], np.cumsum(seg_counts)])
    rank = np.arange(len(src_s)) - seg_start[seg_id]
    core_e = blk_s // bpc
    bl_local = blk_s % bpc

    dnq = np.array([d * 128 for d in dchunks])  # dense slots per (b, q)
    dn_e = dnq[q_s]
    is_dense = rank < dn_e

    # gather idx table (gather slots only)
    gpos = (g_b[g_of_block[bl_local], q_s] + boff_of_block[bl_local] * DN
            + rank - dn_e)
    mg = ~is_dense
    seg_gath = rank[mg] - dn_e[mg]
    assert seg_gath.max() < DN, "gather side overflow"
    idx16 = np.zeros((ncores, 16, totgather // 16), dtype=np.int16)
    idx16[core_e[mg], gpos[mg] % 16, gpos[mg] // 16] = \
        (src_s[mg] - q_s[mg] * qrows).astype(np.int16)
    idx_rep = np.tile(idx16, (1, 8, 1))

    xpad = np.zeros((npad, D_), dtype=np.float32)
    xpad[:n_nodes] = x
    xpad_bf = xpad.astype(ml_dtypes.bfloat16)

    # dense halo tables: [ncores, 128, totdense] bf16;
    # partition p of chunk c holds the row of the dense edge at c*128+p
    densed = np.zeros((ncores, 128, totdense), dtype=ml_dtypes.bfloat16)
    dpos = (d_b[g_of_block[bl_local], q_s]
            + boff_of_block[bl_local] * dn_e + rank)
    md = is_dense
    flat = densed.reshape(ncores * 128, totdense)
    rowsel = core_e[md] * 128 + dpos[md] % 128
    colsel = (dpos[md] // 128) * 128
    flat[rowsel[:, None], colsel[:, None] + np.arange(128)[None, :]] = \
        xpad_bf[src_s[md]]

    # slot one-hot table (same for dense+gather: chunk = rank // 128)
    slot_arr = np.full((ncores, 128, bpc * csum), PAD_SLOT, dtype=np.float32)
    chunk_in_block = qoff[q_s] + rank // 128
    slot_arr[core_e, rank % 128, bl_local * csum + chunk_in_block] = \
        slot_s.astype(np.float32)
    slot_bf = slot_arr.astype(ml_dtypes.bfloat16)

    inv_arr = np.ones((ncores, 128, bpc), dtype=np.float32)
    nodes = np.arange(n_nodes)
    inv_arr[blk_of[nodes] // bpc, slot_of[nodes], blk_of[nodes] % bpc] = inv

    xperm = xpad[perm]

    iota = np.tile(np.arange(SW, dtype=np.float32)[None, :],
                   (128, 1)).astype(ml_dtypes.bfloat16)
    wt = np.ascontiguousarray(W.T.astype(ml_dtypes.bfloat16))

    in_maps = []
    for c in range(ncores):
        in_maps.append({
            "xq": xpad_bf,
            "idxd": np.ascontiguousarray(idx_rep[c]),
            "densed": np.ascontiguousarray(densed[c]),
            "slotd": np.ascontiguousarray(slot_bf[c]),
            "invd": np.ascontiguousarray(inv_arr[c]),
            "xod": np.ascontiguousarray(xperm[c * npc:(c + 1) * npc]),
            "wtd": wt,
            "iotad": iota,
        })
    return in_maps, c_list, groups, bpc, npad, qrows, perm


def kernel(x, src, dst, W, n_nodes=None, trace=False):
    global LAST_RESULTS
    x = np.ascontiguousarray(np.asarray(x, dtype=np.float32))
    W = np.ascontiguousarray(np.asarray(W, dtype=np.float32))
    src = np.asarray(src).astype(np.int64)
    dst = np.asarray(dst).astype(np.int64)
    if n_nodes is None:
        n_nodes = x.shape[0]

    in_maps, c_list, groups, bpc, npad, qrows, perm = _preprocess(
        x, src, dst, W, n_nodes, NCORES)
    nc = _build(tuple(c_list), tuple(groups), bpc, npad, qrows)
    res = bass_utils.run_bass_kernel_spmd(
        nc, in_maps, core_ids=list(range(NCORES)), trace=trace)
    LAST_RESULTS = res
    out_perm = np.concatenate([res.results[c]["out"] for c in range(NCORES)],
                              axis=0)
    out = np.zeros((n_nodes, x.shape[1]), dtype=np.float32)
    valid = perm < n_nodes
    out[perm[valid]] = out_perm[valid]
    return out


# revision 37
# speedup vs baseline: 1.0620x; 1.0620x over previous
"""GCN layer kernel for Trainium2 (8 NeuronCores).

Strategy:
  - Nodes assigned to 8 cores x BPC blocks of 128 via balanced packing so
    each (block, src-quarter) edge segment fits 4*128 edges (minimal pad).
  - Edge features: per (block, quarter) segment of <=512 edges, the first
    256 edge slots are served from a host-built per-block halo table
    (dense, pre-transposed, one HWDGE dma_start per group-quarter); the
    remaining <=256 slots are fetched with gpsimd.dma_gather (SWDGE).
    This halves the SWDGE call count (calls are latency-bound: only 4 fly
    concurrently at ~9.5us each), moving half the traffic to the fast
    dense path per the sharding hint's halo-copy scheme.
  - Segment-sum: per 128-edge chunk, one-hot S[e, slot] built on DVE
    (is_equal vs iota, batched per block), PE matmul accumulates
    psum[feat, slot] += E_bf16^T S over the block's chunks. Pad edges get
    slot 128 (PSUM cols 128..131 ignored).
  - mean+linear+relu+residual: psum1 -> SBUF (ACT copy), f32 matmul with
    W^T, ACT relu with per-partition scale 1/deg (folds the mean), DVE
    residual add, DMA out. deg==0 nodes get a self-edge and scale 1 so
    they keep x (DGL semantics).
"""
import sys
sys.path.insert(0, "/opt/trn_rl_repo")

import numpy as np
import ml_dtypes

import concourse.bass as bass
import concourse.mybir as mybir
import concourse.tile as tile
from concourse import bacc, bass_utils

F32 = mybir.dt.float32
BF16 = mybir.dt.bfloat16
I16 = mybir.dt.int16

N_NODES = 100000
D = 128
NCORES = 8
SW = 132  # slot one-hot width (128 real slots + pad cols)
PAD_SLOT = 128
GATHER_CAP = 1024  # dma_gather crashes above 1024 indices per call
DN = 256  # dense (halo) slots per (block, quarter); rest gathered

_BUILD_CACHE = {}
LAST_RESULTS = None  # for test harness introspection


def _plan_groups(bpc, gmax):
    groups = []
    left = bpc
    while left > 0:
        g = min(gmax, left)
        groups.append(g)
        left -= g
    return groups


def _build(c_list, groups, bpc, npad, qrows):
    """Build + compile the SPMD Bass program. Same program for all 8 cores."""
    key = (tuple(c_list), tuple(groups), bpc, npad, qrows)
    if key in _BUILD_CACHE:
        return _BUILD_CACHE[key]

    assert all(c >= 3 for c in c_list)
    csum = int(sum(c_list))
    dchunks = [c - 2 for c in c_list]  # dense chunks per (block, quarter)
    totdense = bpc * sum(dchunks) * 128
    totgather = bpc * 4 * DN
    npc = bpc * 128  # nodes per core

    nc = bacc.Bacc("TRN2", target_bir_lowering=False, debug=False,
                   num_devices=NCORES, num_swdge_queues=4)
    xq = nc.dram_tensor("xq", [npad, D], BF16, kind="ExternalInput")
    idxd = nc.dram_tensor("idxd", [128, totgather // 16], I16,
                          kind="ExternalInput")
    densed = nc.dram_tensor("densed", [128, totdense], BF16,
                            kind="ExternalInput")
    slotd = nc.dram_tensor("slotd", [128, bpc * csum], BF16,
                           kind="ExternalInput")
    invd = nc.dram_tensor("invd", [128, bpc], F32, kind="ExternalInput")
    xod = nc.dram_tensor("xod", [npc, D], F32, kind="ExternalInput")
    wtd = nc.dram_tensor("wtd", [D, D], BF16, kind="ExternalInput")
    iotad = nc.dram_tensor("iotad", [128, SW], BF16, kind="ExternalInput")
    outd = nc.dram_tensor("out", [npc, D], F32, kind="ExternalOutput")

    qcall = 0  # rotates gather calls over the 4 SWDGE queues

    with tile.TileContext(nc) as tc:
        with tc.tile_pool(name="const", bufs=1) as const, \
             tc.tile_pool(name="ework", bufs=3) as ework, \
             tc.tile_pool(name="sbwork", bufs=3) as sbwork, \
             tc.tile_pool(name="psum1", bufs=2, space="PSUM") as psum1p, \
             tc.tile_pool(name="psum2", bufs=2, space="PSUM") as psum2p:

            wt_t = const.tile([128, D], BF16)
            nc.sync.dma_start(out=wt_t[:], in_=wtd[:, :])
            iota_t = const.tile([128, SW], BF16)
            nc.sync.dma_start(out=iota_t[:], in_=iotad[:, :])
            inv_t = const.tile([128, bpc], F32)
            nc.sync.dma_start(out=inv_t[:], in_=invd[:, :])

            d_base = 0  # (g, q)-major base into densed cols
            g_base = 0  # (g, q)-major base into gather slots
            b0 = 0
            for g_i, G in enumerate(groups):
                slot_t = ework.tile([128, G * csum], BF16, tag="slot")
                nc.sync.dma_start(
                    out=slot_t[:],
                    in_=slotd[:, b0 * csum:(b0 + G) * csum])
                ed_list = []
                eg_list = []
                for q in range(4):
                    nd = G * dchunks[q] * 128
                    ng = G * DN
                    # dense halo part: one contiguous HWDGE load, issued
                    # from the (mostly idle) ACT engine so the big slabs
                    # don't clog the Sync queue ahead of the idx loads
                    ef_d = ework.tile([128, G * dchunks[q], 128], BF16,
                                      tag=f"efd{q}")
                    nc.scalar.dma_start(
                        out=ef_d[:, :, :],
                        in_=densed[:, d_base:d_base + nd])
                    # gather part: one SWDGE call of G*DN (=1024) idxs
                    ncols = ng // 16
                    idx_t = ework.tile([128, ncols], I16, tag=f"idx{q}")
                    nc.sync.dma_start(
                        out=idx_t[:],
                        in_=idxd[:, g_base // 16:
                                 g_base // 16 + ncols])
                    ef_g = ework.tile([128, G * 2, 128], BF16,
                                      tag=f"efg{q}")
                    c0 = 0
                    while c0 * 128 < ng:
                        n_call = min(GATHER_CAP, ng - c0 * 128)
                        nch = n_call // 128
                        nc.gpsimd.dma_gather(
                            out_ap=ef_g[:, c0:c0 + nch, :],
                            in_ap=xq[q * qrows:, :],
                            idxs_ap=idx_t[:, c0 * 8:(c0 + nch) * 8],
                            num_idxs=n_call,
                            num_idxs_reg=n_call,
                            elem_size=D,
                            queue_num=qcall % 4,
                        )
                        qcall += 1
                        c0 += nch
                    ed_list.append(ef_d)
                    eg_list.append(ef_g)
                    d_base += nd
                    g_base += ng

                for bl in range(G):
                    b = b0 + bl
                    s_t = ework.tile([128, csum, SW], BF16, tag="S")
                    slot_sl = slot_t[:, bl * csum:(bl + 1) * csum]
                    nc.vector.tensor_tensor(
                        out=s_t[:, :, :],
                        in0=slot_sl.unsqueeze(2).to_broadcast(
                            [128, csum, SW]),
                        in1=iota_t[:].unsqueeze(1).to_broadcast(
                            [128, csum, SW]),
                        op=mybir.AluOpType.is_equal,
                    )
                    p1 = psum1p.tile([128, SW], F32, tag="p1")
                    cglob = 0
                    for q in range(4):
                        dc = dchunks[q]
                        for k in range(c_list[q]):
                            if k < dc:
                                lhs = ed_list[q][:, bl * dc + k, :]
                            else:
                                lhs = eg_list[q][:, bl * 2 + k - dc, :]
                            nc.tensor.matmul(
                                out=p1[:, :],
                                lhsT=lhs,
                                rhs=s_t[:, cglob, :],
                                start=(cglob == 0),
                                stop=(cglob == csum - 1),
                            )
                            cglob += 1
                    agg_t = sbwork.tile([128, 128], BF16, tag="aggT")
                    nc.scalar.copy(agg_t[:], p1[:, 0:128])
                    p2 = psum2p.tile([128, 128], F32, tag="p2")
                    nc.tensor.matmul(out=p2[:, :], lhsT=agg_t[:],
                                     rhs=wt_t[:], start=True, stop=True)
                    xo_t = sbwork.tile([128, 128], F32, tag="xo")
                    nc.sync.dma_start(out=xo_t[:],
                                      in_=xod[b * 128:(b + 1) * 128, :])
                    hb_t = sbwork.tile([128, 128], F32, tag="hb")
                    nc.scalar.activation(
                        hb_t[:], p2[:, :],
                        mybir.ActivationFunctionType.Relu,
                        scale=inv_t[:, b:b + 1])
                    ob_t = sbwork.tile([128, 128], F32, tag="ob")
                    nc.vector.tensor_add(ob_t[:], hb_t[:], xo_t[:])
                    nc.sync.dma_start(out=outd[b * 128:(b + 1) * 128, :],
                                      in_=ob_t[:])
                b0 += G
    nc.compile()
    _BUILD_CACHE[key] = nc
    return nc


def _pack_blocks(qd, nblocks, cap, node_cap=128):
    """Greedy 4-D balanced packing: assign nodes to blocks so that each
    block's per-quarter edge counts stay <= cap and node count <= node_cap.
    qd: [n, 4] per-node per-quarter in-degree. Returns block id per node,
    or None if infeasible."""
    n = qd.shape[0]
    loads = np.zeros((nblocks, 4), dtype=np.int64)
    slots = np.full(nblocks, node_cap, dtype=np.int64)
    assign = np.full(n, -1, dtype=np.int64)
    order = np.argsort(-qd.sum(1), kind="stable")
    # big nodes first with exact argmin; tail nodes in bulk round-robin
    big = order[qd[order].sum(1) > 24]
    small = order[qd[order].sum(1) <= 24]
    for i in big:
        score = (loads + qd[i]).max(1)
        score[slots <= 0] = 1 << 40
        score[(loads + qd[i] > cap).any(1)] = 1 << 40
        b = int(np.argmin(score))
        if score[b] >= 1 << 40:
            return None
        assign[i] = b
        loads[b] += qd[i]
        slots[b] -= 1
    # small nodes: repeatedly place into least-loaded blocks
    for i in small:
        score = (loads + qd[i]).max(1).astype(np.float64)
        score += (node_cap - slots) * 1e-3  # prefer emptier blocks slightly
        score[slots <= 0] = 1e18
        score[(loads + qd[i] > cap).any(1)] = 1e18
        b = int(np.argmin(score))
        if score[b] >= 1e18:
            return None
        assign[i] = b
        loads[b] += qd[i]
        slots[b] -= 1
    return assign


def _preprocess(x, src, dst, W, n_nodes, ncores, gmax=4, bpc=None):
    """Host-side graph partitioning -> per-core tensors + halo tables."""
    D_ = x.shape[1]
    if bpc is None:
        min_bpc = -(-n_nodes // (ncores * 128))
        bpc = min_bpc + 2 if min_bpc > 8 else min_bpc

    deg = np.bincount(dst, minlength=n_nodes)
    inv = 1.0 / np.maximum(deg, 1).astype(np.float32)
    zero_deg = np.where(deg == 0)[0]
    if len(zero_deg):
        src = np.concatenate([src, zero_deg])
        dst = np.concatenate([dst, zero_deg])

    # Try packed c=[4,4,4,4] at increasing bpc (cap 512 needs mean
    # quarter load comfortably below 512 -> extra block slack), else
    # fall back to natural blocks with data-derived budgets.
    assign = None
    for bpc_try in (bpc + 4, bpc + 8):
        npad_t = bpc_try * 128 * ncores
        qrows_t = npad_t // 4
        quarter_t = (src // qrows_t).astype(np.int64)
        qd = np.zeros((npad_t, 4), dtype=np.int64)
        np.add.at(qd, (dst, quarter_t), 1)
        assign = _pack_blocks(qd, ncores * bpc_try, cap=512)
        if assign is not None:
            bpc = bpc_try
            break
    npc = bpc * 128
    npad = npc * ncores
    qrows = npad // 4
    nblocks = ncores * bpc
    quarter = (src // qrows).astype(np.int64)
    if assign is not None:
        c_list = [4, 4, 4, 4]
    else:
        # fallback: natural-order blocks, budgets from data (+2 gather
        # chunks are carved from each budget, so need c >= 3)
        assign = np.arange(npad) // 128
        cmat = np.zeros((nblocks, 4), dtype=np.int64)
        np.add.at(cmat, (assign[dst], quarter), 1)
        c_list = [max(int(-(-cmat[:, q].max() // 128)), 3) for q in range(4)]
    csum = int(sum(c_list))
    dchunks = [c - 2 for c in c_list]
    qoff = np.concatenate([[0], np.cumsum(c_list)]).astype(int)

    # node -> (block, slot); slot = rank within block
    blk_of = assign[:npad].copy()
    unassigned = np.where(blk_of < 0)[0]
    if len(unassigned):
        counts = np.bincount(blk_of[blk_of >= 0], minlength=nblocks)
        free = []
        for b in range(nblocks):
            free.extend([b] * (128 - counts[b]))
        blk_of[unassigned] = np.array(free[:len(unassigned)], dtype=np.int64)
    order_nodes = np.argsort(blk_of, kind="stable")
    slot_of = np.zeros(npad, dtype=np.int64)
    counts = np.bincount(blk_of, minlength=nblocks)
    assert counts.max() <= 128, "block overflow"
    start = np.concatenate([[0], np.cumsum(counts)])
    slot_of[order_nodes] = np.arange(npad) - start[blk_of[order_nodes]]
    pos_of = blk_of * 128 + slot_of
    perm = np.zeros(npad, dtype=np.int64)
    perm[pos_of] = np.arange(npad)

    groups = _plan_groups(bpc, gmax)
    # (g, q)-major bases for dense slots and gather slots
    d_b = np.zeros((len(groups), 4), dtype=np.int64)
    g_b = np.zeros((len(groups), 4), dtype=np.int64)
    dpos_acc = gpos_acc = 0
    for gi, G in enumerate(groups):
        for q in range(4):
            d_b[gi, q] = dpos_acc
            g_b[gi, q] = gpos_acc
            dpos_acc += G * dchunks[q] * 128
            gpos_acc += G * DN
    totdense, totgather = dpos_acc, gpos_acc
    assert totgather == bpc * 4 * DN

    g_of_block = np.zeros(bpc, dtype=np.int64)
    boff_of_block = np.zeros(bpc, dtype=np.int64)
    b = 0
    for gi, G in enumerate(groups):
        for j in range(G):
            g_of_block[b] = gi
            boff_of_block[b] = j
            b += 1

    # per-edge data
    blk_e = blk_of[dst]
    slot_e = slot_of[dst]
    order = np.lexsort((src, quarter, blk_e))
    src_s = src[order]
    q_s = quarter[order]
    blk_s = blk_e[order]
    slot_s = slot_e[order]

    seg_id = blk_s * 4 + q_s
    seg_counts = np.bincount(seg_id, minlength=nblocks * 4)
    cmat2 = seg_counts.reshape(nblocks, 4)
    for q in range(4):
        assert cmat2[:, q].max() <= c_list[q] * 128, \
            f"quarter {q} overflow: {cmat2[:, q].max()}"
    seg_start = np.concatenate([[0], np.cumsum(seg_counts)])
    rank = np.arange(len(src_s)) - seg_start[seg_id]
    core_e = blk_s // bpc
    bl_local = blk_s % bpc

    dnq = np.array([d * 128 for d in dchunks])  # dense slots per (b, q)
    dn_e = dnq[q_s]
    is_dense = rank < dn_e

    # gather idx table (gather slots only)
    gpos = (g_b[g_of_block[bl_local], q_s] + boff_of_block[bl_local] * DN
            + rank - dn_e)
    mg = ~is_dense
    seg_gath = rank[mg] - dn_e[mg]
    assert seg_gath.max() < DN, "gather side overflow"
    idx16 = np.zeros((ncores, 16, totgather // 16), dtype=np.int16)
    idx16[core_e[mg], gpos[mg] % 16, gpos[mg] // 16] = \
        (src_s[mg] - q_s[mg] * qrows).astype(np.int16)
    idx_rep = np.tile(idx16, (1, 8, 1))

    xpad = np.zeros((npad, D_), dtype=np.float32)
    xpad[:n_nodes] = x
    xpad_bf = xpad.astype(ml_dtypes.bfloat16)

    # dense halo tables: [ncores, 128, totdense] bf16;
    # partition p of chunk c holds the row of the dense edge at c*128+p
    densed = np.zeros((ncores, 128, totdense), dtype=ml_dtypes.bfloat16)
    dpos = (d_b[g_of_block[bl_local], q_s]
            + boff_of_block[bl_local] * dn_e + rank)
    md = is_dense
    flat = densed.reshape(ncores * 128, totdense)
    rowsel = core_e[md] * 128 + dpos[md] % 128
    colsel = (dpos[md] // 128) * 128
    flat[rowsel[:, None], colsel[:, None] + np.arange(128)[None, :]] = \
        xpad_bf[src_s[md]]

    # slot one-hot table (same for dense+gather: chunk = rank // 128)
    slot_arr = np.full((ncores, 128, bpc * csum), PAD_SLOT, dtype=np.float32)
    chunk_in_block = qoff[q_s] + rank // 128
    slot_arr[core_e, rank % 128, bl_local * csum + chunk_in_block] = \
        slot_s.astype(np.float32)
    slot_bf = slot_arr.astype(ml_dtypes.bfloat16)

    inv_arr = np.ones((ncores, 128, bpc), dtype=np.float32)
    nodes = np.arange(n_nodes)
    inv_arr[blk_of[nodes] // bpc, slot_of[nodes], blk_of[nodes] % bpc] = inv

    xperm = xpad[perm]

    iota = np.tile(np.arange(SW, dtype=np.float32)[None, :],
                   (128, 1)).astype(ml_dtypes.bfloat16)
    wt = np.ascontiguousarray(W.T.astype(ml_dtypes.bfloat16))

    in_maps = []
    for c in range(ncores):
        in_maps.append({
            "xq": xpad_bf,
            "idxd": np.ascontiguousarray(idx_rep[c]),
            "densed": np.ascontiguousarray(densed[c]),
            "slotd": np.ascontiguousarray(slot_bf[c]),
            "invd": np.ascontiguousarray(inv_arr[c]),
            "xod": np.ascontiguousarray(xperm[c * npc:(c + 1) * npc]),
            "wtd": wt,
            "iotad": iota,
        })
    return in_maps, c_list, groups, bpc, npad, qrows, perm


def kernel(x, src, dst, W, n_nodes=None, trace=False):
    global LAST_RESULTS
    x = np.ascontiguousarray(np.asarray(x, dtype=np.float32))
    W = np.ascontiguousarray(np.asarray(W, dtype=np.float32))
    src = np.asarray(src).astype(np.int64)
    dst = np.asarray(dst).astype(np.int64)
    if n_nodes is None:
        n_nodes = x.shape[0]

    in_maps, c_list, groups, bpc, npad, qrows, perm = _preprocess(
        x, src, dst, W, n_nodes, NCORES)
    nc = _build(tuple(c_list), tuple(groups), bpc, npad, qrows)
    res = bass_utils.run_bass_kernel_spmd(
        nc, in_maps, core_ids=list(range(NCORES)), trace=trace)
    LAST_RESULTS = res
    out_perm = np.concatenate([res.results[c]["out"] for c in range(NCORES)],
                              axis=0)
    out = np.zeros((n_nodes, x.shape[1]), dtype=np.float32)
    valid = perm < n_nodes
    out[perm[valid]] = out_perm[valid]
    return out


# revision 38
# speedup vs baseline: 1.0665x; 1.0042x over previous
"""GCN layer kernel for Trainium2 (8 NeuronCores).

Strategy:
  - Nodes assigned to 8 cores x BPC blocks of 128 via balanced packing so
    each (block, src-quarter) edge segment fits 4*128 edges (minimal pad).
  - Edge features: per (block, quarter) segment of <=512 edges, the first
    256 edge slots are served from a host-built per-block halo table
    (dense, pre-transposed, one HWDGE dma_start per group-quarter); the
    remaining <=256 slots are fetched with gpsimd.dma_gather (SWDGE).
    This halves the SWDGE call count (calls are latency-bound: only 4 fly
    concurrently at ~9.5us each), moving half the traffic to the fast
    dense path per the sharding hint's halo-copy scheme.
  - Segment-sum: per 128-edge chunk, one-hot S[e, slot] built on DVE
    (is_equal vs iota, batched per block), PE matmul accumulates
    psum[feat, slot] += E_bf16^T S over the block's chunks. Pad edges get
    slot 128 (PSUM cols 128..131 ignored).
  - mean+linear+relu+residual: psum1 -> SBUF (ACT copy), f32 matmul with
    W^T, ACT relu with per-partition scale 1/deg (folds the mean), DVE
    residual add, DMA out. deg==0 nodes get a self-edge and scale 1 so
    they keep x (DGL semantics).
"""
import sys
sys.path.insert(0, "/opt/trn_rl_repo")

import numpy as np
import ml_dtypes

import concourse.bass as bass
import concourse.mybir as mybir
import concourse.tile as tile
from concourse import bacc, bass_utils

F32 = mybir.dt.float32
BF16 = mybir.dt.bfloat16
I16 = mybir.dt.int16

N_NODES = 100000
D = 128
NCORES = 8
SW = 132  # slot one-hot width (128 real slots + pad cols)
PAD_SLOT = 128
GATHER_CAP = 1024  # dma_gather crashes above 1024 indices per call
DN = 256  # dense (halo) slots per (block, quarter); rest gathered

_BUILD_CACHE = {}
LAST_RESULTS = None  # for test harness introspection


def _plan_groups(bpc, gmax):
    groups = []
    left = bpc
    while left > 0:
        g = min(gmax, left)
        groups.append(g)
        left -= g
    return groups


def _build(c_list, groups, bpc, npad, qrows):
    """Build + compile the SPMD Bass program. Same program for all 8 cores."""
    key = (tuple(c_list), tuple(groups), bpc, npad, qrows)
    if key in _BUILD_CACHE:
        return _BUILD_CACHE[key]

    assert all(c >= 3 for c in c_list)
    csum = int(sum(c_list))
    dchunks = [c - 2 for c in c_list]  # dense chunks per (block, quarter)
    totdense = bpc * sum(dchunks) * 128
    totgather = bpc * 4 * DN
    npc = bpc * 128  # nodes per core

    nc = bacc.Bacc("TRN2", target_bir_lowering=False, debug=False,
                   num_devices=NCORES, num_swdge_queues=4)
    xq = nc.dram_tensor("xq", [npad, D], BF16, kind="ExternalInput")
    idxd = nc.dram_tensor("idxd", [128, totgather // 16], I16,
                          kind="ExternalInput")
    densed = nc.dram_tensor("densed", [128, totdense], BF16,
                            kind="ExternalInput")
    slotd = nc.dram_tensor("slotd", [128, bpc * csum], BF16,
                           kind="ExternalInput")
    invd = nc.dram_tensor("invd", [128, bpc], F32, kind="ExternalInput")
    xod = nc.dram_tensor("xod", [npc, D], F32, kind="ExternalInput")
    wtd = nc.dram_tensor("wtd", [D, D], BF16, kind="ExternalInput")
    iotad = nc.dram_tensor("iotad", [128, SW], BF16, kind="ExternalInput")
    outd = nc.dram_tensor("out", [npc, D], F32, kind="ExternalOutput")

    qcall = 0  # rotates gather calls over the 4 SWDGE queues

    with tile.TileContext(nc) as tc:
        with tc.tile_pool(name="const", bufs=1) as const, \
             tc.tile_pool(name="ework", bufs=3) as ework, \
             tc.tile_pool(name="sbwork", bufs=4) as sbwork, \
             tc.tile_pool(name="psum1", bufs=3, space="PSUM") as psum1p, \
             tc.tile_pool(name="psum2", bufs=2, space="PSUM") as psum2p:

            wt_t = const.tile([128, D], BF16)
            nc.sync.dma_start(out=wt_t[:], in_=wtd[:, :])
            iota_t = const.tile([128, SW], BF16)
            nc.sync.dma_start(out=iota_t[:], in_=iotad[:, :])
            inv_t = const.tile([128, bpc], F32)
            nc.sync.dma_start(out=inv_t[:], in_=invd[:, :])

            d_base = 0  # (g, q)-major base into densed cols
            g_base = 0  # (g, q)-major base into gather slots
            b0 = 0
            for g_i, G in enumerate(groups):
                slot_t = ework.tile([128, G * csum], BF16, tag="slot")
                nc.sync.dma_start(
                    out=slot_t[:],
                    in_=slotd[:, b0 * csum:(b0 + G) * csum])
                ed_list = []
                eg_list = []
                for q in range(4):
                    nd = G * dchunks[q] * 128
                    ng = G * DN
                    # dense halo part: one contiguous HWDGE load, issued
                    # from the (mostly idle) ACT engine so the big slabs
                    # don't clog the Sync queue ahead of the idx loads
                    ef_d = ework.tile([128, G * dchunks[q], 128], BF16,
                                      tag=f"efd{q}")
                    nc.scalar.dma_start(
                        out=ef_d[:, :, :],
                        in_=densed[:, d_base:d_base + nd])
                    # gather part: one SWDGE call of G*DN (=1024) idxs
                    ncols = ng // 16
                    idx_t = ework.tile([128, ncols], I16, tag=f"idx{q}")
                    nc.sync.dma_start(
                        out=idx_t[:],
                        in_=idxd[:, g_base // 16:
                                 g_base // 16 + ncols])
                    ef_g = ework.tile([128, G * 2, 128], BF16,
                                      tag=f"efg{q}")
                    c0 = 0
                    while c0 * 128 < ng:
                        n_call = min(GATHER_CAP, ng - c0 * 128)
                        nch = n_call // 128
                        nc.gpsimd.dma_gather(
                            out_ap=ef_g[:, c0:c0 + nch, :],
                            in_ap=xq[q * qrows:, :],
                            idxs_ap=idx_t[:, c0 * 8:(c0 + nch) * 8],
                            num_idxs=n_call,
                            num_idxs_reg=n_call,
                            elem_size=D,
                            queue_num=qcall % 4,
                        )
                        qcall += 1
                        c0 += nch
                    ed_list.append(ef_d)
                    eg_list.append(ef_g)
                    d_base += nd
                    g_base += ng

                for bl in range(G):
                    b = b0 + bl
                    s_t = ework.tile([128, csum, SW], BF16, tag="S")
                    slot_sl = slot_t[:, bl * csum:(bl + 1) * csum]
                    nc.vector.tensor_tensor(
                        out=s_t[:, :, :],
                        in0=slot_sl.unsqueeze(2).to_broadcast(
                            [128, csum, SW]),
                        in1=iota_t[:].unsqueeze(1).to_broadcast(
                            [128, csum, SW]),
                        op=mybir.AluOpType.is_equal,
                    )
                    p1 = psum1p.tile([128, SW], F32, tag="p1")
                    cglob = 0
                    for q in range(4):
                        dc = dchunks[q]
                        for k in range(c_list[q]):
                            if k < dc:
                                lhs = ed_list[q][:, bl * dc + k, :]
                            else:
                                lhs = eg_list[q][:, bl * 2 + k - dc, :]
                            nc.tensor.matmul(
                                out=p1[:, :],
                                lhsT=lhs,
                                rhs=s_t[:, cglob, :],
                                start=(cglob == 0),
                                stop=(cglob == csum - 1),
                            )
                            cglob += 1
                    agg_t = sbwork.tile([128, 128], BF16, tag="aggT")
                    nc.scalar.copy(agg_t[:], p1[:, 0:128])
                    p2 = psum2p.tile([128, 128], F32, tag="p2")
                    nc.tensor.matmul(out=p2[:, :], lhsT=agg_t[:],
                                     rhs=wt_t[:], start=True, stop=True)
                    xo_t = sbwork.tile([128, 128], F32, tag="xo")
                    nc.sync.dma_start(out=xo_t[:],
                                      in_=xod[b * 128:(b + 1) * 128, :])
                    hb_t = sbwork.tile([128, 128], F32, tag="hb")
                    nc.scalar.activation(
                        hb_t[:], p2[:, :],
                        mybir.ActivationFunctionType.Relu,
                        scale=inv_t[:, b:b + 1])
                    ob_t = sbwork.tile([128, 128], F32, tag="ob")
                    nc.vector.tensor_add(ob_t[:], hb_t[:], xo_t[:])
                    nc.sync.dma_start(out=outd[b * 128:(b + 1) * 128, :],
                                      in_=ob_t[:])
                b0 += G
    nc.compile()
    _BUILD_CACHE[key] = nc
    return nc


def _pack_blocks(qd, nblocks, cap, node_cap=128):
    """Greedy 4-D balanced packing: assign nodes to blocks so that each
    block's per-quarter edge counts stay <= cap and node count <= node_cap.
    qd: [n, 4] per-node per-quarter in-degree. Returns block id per node,
    or None if infeasible."""
    n = qd.shape[0]
    loads = np.zeros((nblocks, 4), dtype=np.int64)
    slots = np.full(nblocks, node_cap, dtype=np.int64)
    assign = np.full(n, -1, dtype=np.int64)
    order = np.argsort(-qd.sum(1), kind="stable")
    # big nodes first with exact argmin; tail nodes in bulk round-robin
    big = order[qd[order].sum(1) > 24]
    small = order[qd[order].sum(1) <= 24]
    for i in big:
        score = (loads + qd[i]).max(1)
        score[slots <= 0] = 1 << 40
        score[(loads + qd[i] > cap).any(1)] = 1 << 40
        b = int(np.argmin(score))
        if score[b] >= 1 << 40:
            return None
        assign[i] = b
        loads[b] += qd[i]
        slots[b] -= 1
    # small nodes: repeatedly place into least-loaded blocks
    for i in small:
        score = (loads + qd[i]).max(1).astype(np.float64)
        score += (node_cap - slots) * 1e-3  # prefer emptier blocks slightly
        score[slots <= 0] = 1e18
        score[(loads + qd[i] > cap).any(1)] = 1e18
        b = int(np.argmin(score))
        if score[b] >= 1e18:
            return None
        assign[i] = b
        loads[b] += qd[i]
        slots[b] -= 1
    return assign


def _preprocess(x, src, dst, W, n_nodes, ncores, gmax=4, bpc=None):
    """Host-side graph partitioning -> per-core tensors + halo tables."""
    D_ = x.shape[1]
    if bpc is None:
        min_bpc = -(-n_nodes // (ncores * 128))
        bpc = min_bpc + 2 if min_bpc > 8 else min_bpc

    deg = np.bincount(dst, minlength=n_nodes)
    inv = 1.0 / np.maximum(deg, 1).astype(np.float32)
    zero_deg = np.where(deg == 0)[0]
    if len(zero_deg):
        src = np.concatenate([src, zero_deg])
        dst = np.concatenate([dst, zero_deg])

    # Try packed c=[4,4,4,4] at increasing bpc (cap 512 needs mean
    # quarter load comfortably below 512 -> extra block slack), else
    # fall back to natural blocks with data-derived budgets.
    assign = None
    for bpc_try in (bpc + 4, bpc + 8):
        npad_t = bpc_try * 128 * ncores
        qrows_t = npad_t // 4
        quarter_t = (src // qrows_t).astype(np.int64)
        qd = np.zeros((npad_t, 4), dtype=np.int64)
        np.add.at(qd, (dst, quarter_t), 1)
        assign = _pack_blocks(qd, ncores * bpc_try, cap=512)
        if assign is not None:
            bpc = bpc_try
            break
    npc = bpc * 128
    npad = npc * ncores
    qrows = npad // 4
    nblocks = ncores * bpc
    quarter = (src // qrows).astype(np.int64)
    if assign is not None:
        c_list = [4, 4, 4, 4]
    else:
        # fallback: natural-order blocks, budgets from data (+2 gather
        # chunks are carved from each budget, so need c >= 3)
        assign = np.arange(npad) // 128
        cmat = np.zeros((nblocks, 4), dtype=np.int64)
        np.add.at(cmat, (assign[dst], quarter), 1)
        c_list = [max(int(-(-cmat[:, q].max() // 128)), 3) for q in range(4)]
    csum = int(sum(c_list))
    dchunks = [c - 2 for c in c_list]
    qoff = np.concatenate([[0], np.cumsum(c_list)]).astype(int)

    # node -> (block, slot); slot = rank within block
    blk_of = assign[:npad].copy()
    unassigned = np.where(blk_of < 0)[0]
    if len(unassigned):
        counts = np.bincount(blk_of[blk_of >= 0], minlength=nblocks)
        free = []
        for b in range(nblocks):
            free.extend([b] * (128 - counts[b]))
        blk_of[unassigned] = np.array(free[:len(unassigned)], dtype=np.int64)
    order_nodes = np.argsort(blk_of, kind="stable")
    slot_of = np.zeros(npad, dtype=np.int64)
    counts = np.bincount(blk_of, minlength=nblocks)
    assert counts.max() <= 128, "block overflow"
    start = np.concatenate([[0], np.cumsum(counts)])
    slot_of[order_nodes] = np.arange(npad) - start[blk_of[order_nodes]]
    pos_of = blk_of * 128 + slot_of
    perm = np.zeros(npad, dtype=np.int64)
    perm[pos_of] = np.arange(npad)

    groups = _plan_groups(bpc, gmax)
    # (g, q)-major bases for dense slots and gather slots
    d_b = np.zeros((len(groups), 4), dtype=np.int64)
    g_b = np.zeros((len(groups), 4), dtype=np.int64)
    dpos_acc = gpos_acc = 0
    for gi, G in enumerate(groups):
        for q in range(4):
            d_b[gi, q] = dpos_acc
            g_b[gi, q] = gpos_acc
            dpos_acc += G * dchunks[q] * 128
            gpos_acc += G * DN
    totdense, totgather = dpos_acc, gpos_acc
    assert totgather == bpc * 4 * DN

    g_of_block = np.zeros(bpc, dtype=np.int64)
    boff_of_block = np.zeros(bpc, dtype=np.int64)
    b = 0
    for gi, G in enumerate(groups):
        for j in range(G):
            g_of_block[b] = gi
            boff_of_block[b] = j
            b += 1

    # per-edge data
    blk_e = blk_of[dst]
    slot_e = slot_of[dst]
    order = np.lexsort((src, quarter, blk_e))
    src_s = src[order]
    q_s = quarter[order]
    blk_s = blk_e[order]
    slot_s = slot_e[order]

    seg_id = blk_s * 4 + q_s
    seg_counts = np.bincount(seg_id, minlength=nblocks * 4)
    cmat2 = seg_counts.reshape(nblocks, 4)
    for q in range(4):
        assert cmat2[:, q].max() <= c_list[q] * 128, \
            f"quarter {q} overflow: {cmat2[:, q].max()}"
    seg_start = np.concatenate([[0], np.cumsum(seg_counts)])
    rank = np.arange(len(src_s)) - seg_start[seg_id]
    core_e = blk_s // bpc
    bl_local = blk_s % bpc

    dnq = np.array([d * 128 for d in dchunks])  # dense slots per (b, q)
    dn_e = dnq[q_s]
    is_dense = rank < dn_e

    # gather idx table (gather slots only)
    gpos = (g_b[g_of_block[bl_local], q_s] + boff_of_block[bl_local] * DN
            + rank - dn_e)
    mg = ~is_dense
    seg_gath = rank[mg] - dn_e[mg]
    assert seg_gath.max() < DN, "gather side overflow"
    idx16 = np.zeros((ncores, 16, totgather // 16), dtype=np.int16)
    idx16[core_e[mg], gpos[mg] % 16, gpos[mg] // 16] = \
        (src_s[mg] - q_s[mg] * qrows).astype(np.int16)
    idx_rep = np.tile(idx16, (1, 8, 1))

    xpad = np.zeros((npad, D_), dtype=np.float32)
    xpad[:n_nodes] = x
    xpad_bf = xpad.astype(ml_dtypes.bfloat16)

    # dense halo tables: [ncores, 128, totdense] bf16;
    # partition p of chunk c holds the row of the dense edge at c*128+p
    densed = np.zeros((ncores, 128, totdense), dtype=ml_dtypes.bfloat16)
    dpos = (d_b[g_of_block[bl_local], q_s]
            + boff_of_block[bl_local] * dn_e + rank)
    md = is_dense
    flat = densed.reshape(ncores * 128, totdense)
    rowsel = core_e[md] * 128 + dpos[md] % 128
    colsel = (dpos[md] // 128) * 128
    flat[rowsel[:, None], colsel[:, None] + np.arange(128)[None, :]] = \
        xpad_bf[src_s[md]]

    # slot one-hot table (same for dense+gather: chunk = rank // 128)
    slot_arr = np.full((ncores, 128, bpc * csum), PAD_SLOT, dtype=np.float32)
    chunk_in_block = qoff[q_s] + rank // 128
    slot_arr[core_e, rank % 128, bl_local * csum + chunk_in_block] = \
        slot_s.astype(np.float32)
    slot_bf = slot_arr.astype(ml_dtypes.bfloat16)

    inv_arr = np.ones((ncores, 128, bpc), dtype=np.float32)
    nodes = np.arange(n_nodes)
    inv_arr[blk_of[nodes] // bpc, slot_of[nodes], blk_of[nodes] % bpc] = inv

    xperm = xpad[perm]

    iota = np.tile(np.arange(SW, dtype=np.float32)[None, :],
                   (128, 1)).astype(ml_dtypes.bfloat16)
    wt = np.ascontiguousarray(W.T.astype(ml_dtypes.bfloat16))

    in_maps = []
    for c in range(ncores):
        in_maps.append({
            "xq": xpad_bf,
            "idxd": np.ascontiguousarray(idx_rep[c]),
            "densed": np.ascontiguousarray(densed[c]),
            "slotd": np.ascontiguousarray(slot_bf[c]),
            "invd": np.ascontiguousarray(inv_arr[c]),
            "xod": np.ascontiguousarray(xperm[c * npc:(c + 1) * npc]),
            "wtd": wt,
            "iotad": iota,
        })
    return in_maps, c_list, groups, bpc, npad, qrows, perm


def kernel(x, src, dst, W, n_nodes=None, trace=False):
    global LAST_RESULTS
    x = np.ascontiguousarray(np.asarray(x, dtype=np.float32))
    W = np.ascontiguousarray(np.asarray(W, dtype=np.float32))
    src = np.asarray(src).astype(np.int64)
    dst = np.asarray(dst).astype(np.int64)
    if n_nodes is None:
        n_nodes = x.shape[0]

    in_maps, c_list, groups, bpc, npad, qrows, perm = _preprocess(
        x, src, dst, W, n_nodes, NCORES)
    nc = _build(tuple(c_list), tuple(groups), bpc, npad, qrows)
    res = bass_utils.run_bass_kernel_spmd(
        nc, in_maps, core_ids=list(range(NCORES)), trace=trace)
    LAST_RESULTS = res
    out_perm = np.concatenate([res.results[c]["out"] for c in range(NCORES)],
                              axis=0)
    out = np.zeros((n_nodes, x.shape[1]), dtype=np.float32)
    valid = perm < n_nodes
    out[perm[valid]] = out_perm[valid]
    return out
